# revision 9
# baseline (speedup 1.0000x reference)
"""EnergyTransformer Trainium2 kernel: 8-core data-parallel over batch.

Each core handles 2 images. State h kept d-major [768, 2*197] in SBUF fp32.
Matmuls in bf16 (1 cyc/row); layernorm stats via fp32r ones-matmuls.
"""
import numpy as np
import ml_dtypes
from contextlib import ExitStack

import concourse.bass as bass
import concourse.tile as tile
from concourse import bacc, mybir
from concourse.bass_utils import run_bass_kernel_spmd

BF16 = mybir.dt.bfloat16
F32 = mybir.dt.float32
F32R = mybir.dt.float32r
AF = mybir.ActivationFunctionType

D = 768            # embed dim
NT = 197           # tokens per image (incl cls)
NI = 2             # images per core
T = NI * NT        # 394 token columns
HOP = 3072
NHEAD = 12
HDIM = 64
NSTEP = 6
BETA = 1.0 / 8.0
EPS = 1e-5
P = 128
DC = D // P        # 6 d-chunks
HC = HOP // P      # 24 hopfield chunks

# token chunks: per image [128, 69]
TOK_CHUNKS = [(im, mc, im * NT + mc * P, 128 if mc == 0 else NT - P)
              for im in range(NI) for mc in range(2)]  # (img, mc, col0, valid)

_CACHE = {}


def _build():
    nc = bacc.Bacc("TRN2", target_bir_lowering=False, debug=False, num_devices=8)

    # ---- DRAM params (per-core) ----
    patches = nc.declare_dram_parameter("patches", [D, 512], F32R, isOutput=False)
    h0T = nc.declare_dram_parameter("h0T", [D, T], F32, isOutput=False)
    pw = nc.declare_dram_parameter("pw", [D, D], F32R, isOutput=False)
    wq_d = nc.declare_dram_parameter("wq_d", [D, D], BF16, isOutput=False)
    wk_d = nc.declare_dram_parameter("wk_d", [D, D], BF16, isOutput=False)
    w_rot = nc.declare_dram_parameter("w_rot", [2 * D, D], BF16, isOutput=False)
    xi_d = nc.declare_dram_parameter("xi_d", [D, HOP], BF16, isOutput=False)
    xi_T = nc.declare_dram_parameter("xi_T", [HOP, D], BF16, isOutput=False)
    out_w = nc.declare_dram_parameter("out_w", [D, D], BF16, isOutput=False)
    lng = nc.declare_dram_parameter("lng", [D], F32, isOutput=False)
    lnb = nc.declare_dram_parameter("lnb", [D], F32, isOutput=False)
    outg = nc.declare_dram_parameter("outg", [D], F32, isOutput=False)
    outbeta = nc.declare_dram_parameter("outbeta", [D], F32, isOutput=False)
    out_b = nc.declare_dram_parameter("out_b", [D], F32, isOutput=False)
    out = nc.declare_dram_parameter("out", [NI, NT - 1, D], F32, isOutput=True)

    with tile.TileContext(nc) as tc, ExitStack() as ctx:
        const = ctx.enter_context(tc.tile_pool(name="const", bufs=1))
        state = ctx.enter_context(tc.tile_pool(name="state", bufs=1))
        bigw = ctx.enter_context(tc.tile_pool(name="bigw", bufs=1))
        p_patch = ctx.enter_context(tc.tile_pool(name="p_patch", bufs=6))
        p_sq = ctx.enter_context(tc.tile_pool(name="p_sq", bufs=6))
        p_hr = ctx.enter_context(tc.tile_pool(name="p_hr", bufs=6))
        p_row = ctx.enter_context(tc.tile_pool(name="p_row", bufs=4))
        p_R = ctx.enter_context(tc.tile_pool(name="p_R", bufs=8))
        p_RT = ctx.enter_context(tc.tile_pool(name="p_RT", bufs=5))
        p_st = ctx.enter_context(tc.tile_pool(name="p_st", bufs=5))
        p_ln = ctx.enter_context(tc.tile_pool(name="p_ln", bufs=5))
        p_hid = ctx.enter_context(tc.tile_pool(name="p_hid", bufs=3))
        p_xiT = ctx.enter_context(tc.tile_pool(name="p_xiT", bufs=6))
        p_out = ctx.enter_context(tc.tile_pool(name="p_out", bufs=2))
        ps = ctx.enter_context(tc.tile_pool(name="ps", bufs=8, space="PSUM"))

        def psum(pp=P, ff=512):
            t = ps.tile([pp, ff], F32, tag="ps", name="pst")
            return t[:pp, :ff]

        # ---- constants / weights resident ----
        wq_t = const.tile([P, DC, D], BF16)
        nc.sync.dma_start(wq_t[:], wq_d.rearrange("(o p) c -> p o c", p=P))
        wk_t = const.tile([P, DC, D], BF16)
        nc.sync.dma_start(wk_t[:], wk_d.rearrange("(o p) c -> p o c", p=P))
        wrot_t = const.tile([P, 2 * DC, D], BF16)
        nc.sync.dma_start(wrot_t[:], w_rot.rearrange("(o p) d -> p o d", p=P))
        xi_t = const.tile([P, DC, HOP], BF16)
        nc.sync.dma_start(xi_t[:], xi_d.rearrange("(o p) j -> p o j", p=P))
        lng_t = const.tile([P, DC], F32)
        nc.sync.dma_start(lng_t[:], lng.rearrange("(o p) -> p o", p=P))
        lnb_t = const.tile([P, DC], F32)
        nc.sync.dma_start(lnb_t[:], lnb.rearrange("(o p) -> p o", p=P))
        outg_t = const.tile([P, DC], F32)
        nc.sync.dma_start(outg_t[:], outg.rearrange("(o p) -> p o", p=P))
        outbeta_t = const.tile([P, DC], F32)
        nc.sync.dma_start(outbeta_t[:], outbeta.rearrange("(o p) -> p o", p=P))
        outb_row = const.tile([1, D], F32)
        nc.sync.dma_start(outb_row[:], out_b[None, :])

        ones_colf = const.tile([P, 1], F32)
        nc.vector.memset(ones_colf[:], 1.0)
        ones_col = const.tile([P, 1], F32R)
        nc.vector.tensor_copy(ones_col[:], ones_colf[:])
        ones_row = const.tile([1, P], F32)
        nc.vector.memset(ones_row[:], 1.0)
        neg8_row = const.tile([1, P], BF16)
        nc.vector.memset(neg8_row[:], -8.0)
        ident = const.tile([P, P], F32)
        from concourse.masks import make_identity
        make_identity(nc, ident[:])
        eps_t = const.tile([1, 1], F32)
        nc.vector.memset(eps_t[:], EPS)

        # ---- state ----
        HT = state.tile([P, DC, T], F32)       # h d-major
        gT = state.tile([P, DC, T], BF16)      # LN(h) d-major
        QT = state.tile([P, DC, T], BF16)      # q^T (head-major rows)
        KT = state.tile([P, DC, T], BF16)
        Qtok = state.tile([P, 4, D], BF16)     # q token-major (4 tok chunks)
        Ktok = state.tile([P, 4, D], BF16)
        UVT = state.tile([P, 2 * DC, T], BF16)  # [U^T; V^T]

        # ---- pre-stage: h0 = bias ; h0 += patches @ patch_w ----
        nc.sync.dma_start(HT[:], h0T.rearrange("(o p) t -> p o t", p=P))
        pw_t = bigw.tile([P, DC, D], F32R, tag="bigw")
        nc.sync.dma_start(pw_t[:], pw.rearrange("(o p) d -> p o d", p=P))
        for im in range(NI):
            pch = []
            for k in range(DC):
                t = p_patch.tile([P, 256], F32R, tag="patch")
                nc.sync.dma_start(t[:], patches[k * P:(k + 1) * P,
                                               im * 256:(im + 1) * 256])
                pch.append(t)
            for i in range(DC):
                pp = psum(P, 256)
                for k in range(DC):
                    nc.tensor.matmul(pp, pw_t[:, k, i * P:(i + 1) * P],
                                     pch[k][:],
                                     start=(k == 0), stop=(k == DC - 1))
                cc = im * NT + 1
                nc.vector.tensor_add(HT[:, i, cc:cc + NT - 1], HT[:, i, cc:cc + NT - 1],
                                     pp[:, 0:NT - 1])

        # ---- helpers ----
        def layernorm(gamma_t, beta_t, out_tile):
            hrs, sqs = [], []
            for i in range(DC):
                hr = p_hr.tile([P, T], F32R, tag="hr", name="hr")
                nc.vector.tensor_copy(hr[:], HT[:, i, :])
                hrs.append(hr)
                sq = p_sq.tile([P, T], F32R, tag="sq")
                nc.vector.tensor_mul(sq[:], hr[:], hr[:])
                sqs.append(sq)
            mu_ps = psum(1, T)
            for i in range(DC):
                nc.tensor.matmul(mu_ps, ones_col[:], hrs[i][:],
                                 start=(i == 0), stop=(i == DC - 1))
            ss_ps = psum(1, T)
            for i in range(DC):
                nc.tensor.matmul(ss_ps, ones_col[:], sqs[i][:],
                                 start=(i == 0), stop=(i == DC - 1))
            t_mu = p_row.tile([1, T], F32, tag="row")
            nc.vector.tensor_scalar_mul(t_mu[:], mu_ps[0:1, :], 1.0 / D)
            t_var = p_row.tile([1, T], F32, tag="row")
            nc.vector.tensor_scalar_mul(t_var[:], ss_ps[0:1, :], 1.0 / D)
            t_ms = p_row.tile([1, T], F32, tag="row")
            nc.vector.tensor_mul(t_ms[:], t_mu[:], t_mu[:])
            nc.vector.tensor_sub(t_var[:], t_var[:], t_ms[:])
            nc.scalar.activation(t_var[:], t_var[:], AF.Sqrt, bias=eps_t[0:1, :])
            a_row = p_row.tile([1, T], F32, tag="row")
            nc.vector.reciprocal(a_row[:], t_var[:])
            b_row = t_ms
            nc.vector.tensor_mul(b_row[:], t_mu[:], a_row[:])
            nc.vector.tensor_scalar_mul(b_row[:], b_row[:], -1.0)
            a_ps = psum(P, T)
            nc.tensor.matmul(a_ps, ones_row[0:1, :], a_row[0:1, :],
                             start=True, stop=True)
            b_ps = psum(P, T)
            nc.tensor.matmul(b_ps, ones_row[0:1, :], b_row[0:1, :],
                             start=True, stop=True)
            for i in range(DC):
                t = p_hr.tile([P, T], F32, tag="gtmp", name="gtmp")
                nc.vector.tensor_tensor(t[:], HT[:, i, :], a_ps[:, :],
                                        mybir.AluOpType.mult)
                nc.vector.tensor_tensor(t[:], t[:], b_ps[:, :],
                                        mybir.AluOpType.add)
                nc.vector.tensor_scalar(out_tile[:, i, :], t[:],
                                        gamma_t[:, i:i + 1], beta_t[:, i:i + 1],
                                        mybir.AluOpType.mult, mybir.AluOpType.add)

        # ---- 6 time steps ----
        for step in range(NSTEP):
            layernorm(lng_t, lnb_t, gT)

            # QT/KT d-major: for chunk i, rows = heads (2i, 2i+1)
            for (w_t, dst) in ((wq_t, QT), (wk_t, KT)):
                for i in range(DC):
                    pp = psum(P, T)
                    for k in range(DC):
                        nc.tensor.matmul(pp, w_t[:, k, i * P:(i + 1) * P],
                                         gT[:, k, :],
                                         start=(k == 0), stop=(k == DC - 1))
                    nc.scalar.copy(dst[:, i, :], pp[:, :])

            # Qtok/Ktok token-major
            for (w_t, dst) in ((wq_t, Qtok), (wk_t, Ktok)):
                for tci, (im, mc, c0, vm) in enumerate(TOK_CHUNKS):
                    for b in range(2):
                        pp = psum(vm, 384)
                        for k in range(DC):
                            nc.tensor.matmul(pp, gT[:, k, c0:c0 + vm],
                                             w_t[:, k, b * 384:(b + 1) * 384],
                                             start=(k == 0), stop=(k == DC - 1))
                        nc.vector.tensor_copy(dst[:vm, tci, b * 384:(b + 1) * 384], pp)

            # attention: 12 (img, head-pair) units, software-pipelined lag 1
            pairs = [(im, hp) for im in range(NI) for hp in range(DC)]
            saved = {}

            def emit_A(pidx):
                im, hp = pairs[pidx]
                ic0 = im * NT
                Rt, st2 = {}, {}
                for mc in range(2):
                    c0 = ic0 + mc * P
                    vm = 128 if mc == 0 else NT - P
                    st2[mc] = p_st.tile([P, 2], F32, tag="st2", name="st2")
                    for hh in range(2):
                        off = hh * HDIM
                        pa = psum(vm, NT)
                        nc.tensor.matmul(pa, QT[off:off + HDIM, hp, c0:c0 + vm],
                                         KT[off:off + HDIM, hp, ic0:ic0 + NT],
                                         start=True, stop=True)
                        E = p_R.tile([P, NT], BF16, tag="R")
                        nc.scalar.activation(E[:vm, :], pa, AF.Exp, scale=BETA,
                                             accum_out=st2[mc][:vm, hh:hh + 1])
                        Rt[(hh, mc)] = E
                    rd = p_st.tile([P, 2], F32, tag="rd")
                    nc.vector.reciprocal(rd[:vm, :], st2[mc][:vm, :])
                    for hh in range(2):
                        nc.vector.tensor_scalar_mul(Rt[(hh, mc)][:vm, :],
                                                    Rt[(hh, mc)][:vm, :],
                                                    rd[:vm, hh:hh + 1])
                saved[pidx] = (Rt, st2)

            def emit_rest(pidx):
                im, hp = pairs[pidx]
                ic0 = im * NT
                Rt, st2 = saved.pop(pidx)
                # stats transpose -> ln rows
                lnr = [p_ln.tile([1, NT], BF16, tag="lnr", name="lnr") for _ in range(2)]
                for mc in range(2):
                    vm = 128 if mc == 0 else NT - P
                    for hh in range(2):
                        pt = psum(1, P)
                        nc.tensor.transpose(pt[:1, :vm], st2[mc][:vm, hh:hh + 1],
                                            ident[:vm, :vm])
                        nc.scalar.activation(lnr[hh][0:1, mc * P:mc * P + vm],
                                             pt[0:1, :vm], AF.Ln,
                                             scale=1.0 / NT)
                # V^T = sum_mc Qtok_slice.T @ R
                for hh in range(2):
                    off = hh * HDIM
                    pv = psum(HDIM, NT)
                    for mc in range(2):
                        vm = 128 if mc == 0 else NT - P
                        tci = im * 2 + mc
                        nc.tensor.matmul(pv, Qtok[:vm, tci, hp * P + off:hp * P + off + HDIM],
                                         Rt[(hh, mc)][:vm, :],
                                         start=(mc == 0), stop=(mc == 1))
                    nc.vector.tensor_copy(UVT[off:off + HDIM, DC + hp, ic0:ic0 + NT], pv)
                # B scores + rank1(-8*lnsum) -> exp -> R^T ; U^T
                for hh in range(2):
                    off = hh * HDIM
                    RTs = []
                    for mc in range(2):
                        vn = 128 if mc == 0 else NT - P
                        c0 = ic0 + mc * P
                        pb = psum(vn, NT)
                        nc.tensor.matmul(pb, KT[off:off + HDIM, hp, c0:c0 + vn],
                                         QT[off:off + HDIM, hp, ic0:ic0 + NT],
                                         start=True, stop=False)
                        nc.tensor.matmul(pb, neg8_row[0:1, 0:vn], lnr[hh][0:1, :],
                                         start=False, stop=True)
                        RTt = p_RT.tile([P, NT], BF16, tag="RT")
                        nc.scalar.activation(RTt[:vn, :], pb, AF.Exp, scale=BETA)
                        RTs.append((RTt, vn))
                    pu = psum(HDIM, NT)
                    for mc in range(2):
                        RTt, vn = RTs[mc]
                        tci = im * 2 + mc
                        nc.tensor.matmul(pu, Ktok[:vn, tci, hp * P + off:hp * P + off + HDIM],
                                         RTt[:vn, :],
                                         start=(mc == 0), stop=(mc == 1))
                    nc.vector.tensor_scalar_mul(UVT[off:off + HDIM, hp, ic0:ic0 + NT],
                                                pu, 1.0 / NT)

            for idx in range(len(pairs) + 1):
                if idx < len(pairs):
                    emit_A(idx)
                if idx >= 1:
                    emit_rest(idx - 1)

            # delta_attn^T accumulate (6 banks held)
            dps = []
            for i in range(DC):
                pp = psum(P, T)
                for j in range(2 * DC):
                    nc.tensor.matmul(pp, wrot_t[:, j, i * P:(i + 1) * P],
                                     UVT[:, j, :],
                                     start=(j == 0), stop=(j == 2 * DC - 1))
                dps.append(pp)
            for i in range(DC):
                nc.vector.tensor_add(HT[:, i, :], HT[:, i, :], dps[i][:, :])

            # hopfield fwd+bwd fused; 6 accumulators
            hps = [psum(P, T) for _ in range(DC)]
            for c in range(HC):
                xiTt = p_xiT.tile([P, D], BF16, tag="xiT")
                nc.sync.dma_start(xiTt[:],
                                  xi_T.rearrange("(o p) d -> p o d", p=P)[:, c, :])
                pf = psum(P, T)
                for k in range(DC):
                    nc.tensor.matmul(pf, xi_t[:, k, c * P:(c + 1) * P],
                                     gT[:, k, :],
                                     start=(k == 0), stop=(k == DC - 1))
                hid = p_hid.tile([P, T], BF16, tag="hid")
                nc.scalar.activation(hid[:], pf, AF.Relu)
                for i in range(DC):
                    nc.tensor.matmul(hps[i], xiTt[:, i * P:(i + 1) * P], hid[:],
                                     start=(c == 0), stop=(c == HC - 1))
            for i in range(DC):
                nc.vector.tensor_add(HT[:, i, :], HT[:, i, :], hps[i][:, :])

        # ---- output stage ----
        layernorm(outg_t, outbeta_t, gT)
        ow_t = bigw.tile([P, DC, D], BF16, tag="bigw")
        nc.sync.dma_start(ow_t[:], out_w.rearrange("(o p) d -> p o d", p=P))
        obb = const.tile([P, D], F32, tag="obb", name="obb")
        for b in range(2):
            pp = psum(P, 384)
            nc.tensor.matmul(pp, ones_row[0:1, :], outb_row[0:1, b * 384:(b + 1) * 384],
                             start=True, stop=True)
            nc.vector.tensor_copy(obb[:, b * 384:(b + 1) * 384], pp)
        for tci, (im, mc, c0, vm) in enumerate(TOK_CHUNKS):
            osb = p_out.tile([P, D], F32, tag="osb")
            for b in range(2):
                pp = psum(vm, 384)
                for k in range(DC):
                    nc.tensor.matmul(pp, gT[:, k, c0:c0 + vm],
                                     ow_t[:, k, b * 384:(b + 1) * 384],
                                     start=(k == 0), stop=(k == DC - 1))
                nc.vector.tensor_tensor(osb[:vm, b * 384:(b + 1) * 384], pp,
                                        obb[:vm, b * 384:(b + 1) * 384],
                                        mybir.AluOpType.add)
            if mc == 0:
                nc.sync.dma_start(out[im, 0:P - 1, :], osb[1:P, :])
            else:
                nc.sync.dma_start(out[im, P - 1:NT - 1, :], osb[0:vm, :])

    nc.finalize()
    return nc


def _get_nc():
    if "nc" not in _CACHE:
        _CACHE["nc"] = _build()
    return _CACHE["nc"]


def kernel(**inputs):
    x = np.asarray(inputs["x"], dtype=np.float32)            # [16,3,224,224]
    patch_w = np.asarray(inputs["patch_w"], dtype=np.float32)
    patch_b = np.asarray(inputs["patch_b"], dtype=np.float32)
    cls_token = np.asarray(inputs["cls_token"], dtype=np.float32)
    pos_embed = np.asarray(inputs["pos_embed"], dtype=np.float32)
    ln_gamma = np.asarray(inputs["ln_gamma"], dtype=np.float32)
    ln_beta = np.asarray(inputs["ln_beta"], dtype=np.float32)
    wq = np.asarray(inputs["wq"], dtype=np.float32)
    wk = np.asarray(inputs["wk"], dtype=np.float32)
    xi = np.asarray(inputs["xi"], dtype=np.float32)
    out_gamma = np.asarray(inputs["out_gamma"], dtype=np.float32)
    out_beta = np.asarray(inputs["out_beta"], dtype=np.float32)
    out_w = np.asarray(inputs["out_w"], dtype=np.float32)
    out_b = np.asarray(inputs["out_b"], dtype=np.float32)

    B = x.shape[0]
    bf = ml_dtypes.bfloat16
    # host-side prep (shared across cores)
    wq_d = np.ascontiguousarray(wq.transpose(1, 0, 2).reshape(D, D)).astype(bf)
    wk_d = np.ascontiguousarray(wk.transpose(1, 0, 2).reshape(D, D)).astype(bf)
    w_rot = np.concatenate([wq.transpose(0, 2, 1).reshape(D, D),
                            wk.transpose(0, 2, 1).reshape(D, D)], axis=0).astype(bf)
    xi_b = xi.astype(bf)
    xi_Tb = np.ascontiguousarray(xi.T).astype(bf)
    out_wb = out_w.astype(bf)

    # h0 bias columns: [768, 394]
    pos = pos_embed[0]                                        # [197, 768]
    h0 = np.empty((NT, D), dtype=np.float32)
    h0[0] = cls_token[0, 0] + pos[0]
    h0[1:] = pos[1:] + patch_b[None, :]
    h0T_one = np.ascontiguousarray(h0.T)                      # [768, 197]
    h0T = np.concatenate([h0T_one, h0T_one], axis=1)          # [768, 394]

    # patchify (host): [B, 196, 768]
    xr = x.reshape(B, 3, 14, 16, 14, 16).transpose(0, 2, 4, 1, 3, 5).reshape(B, 196, D)

    def tf32_round(a):
        b = a.astype(np.float32).copy()
        b.view(np.uint32)[...] &= np.uint32(0xFFFFE000)
        return b

    patch_w_r = tf32_round(patch_w)
    nc = _get_nc()
    in_maps = []
    for c in range(8):
        pT = np.zeros((D, 512), dtype=np.float32)
        pT[:, 0:196] = xr[2 * c].T
        pT[:, 256:452] = xr[2 * c + 1].T
        in_maps.append({
            "patches": tf32_round(pT), "h0T": h0T, "pw": patch_w_r,
            "wq_d": wq_d, "wk_d": wk_d, "w_rot": w_rot,
            "xi_d": xi_b, "xi_T": xi_Tb, "out_w": out_wb,
            "lng": ln_gamma, "lnb": ln_beta, "outg": out_gamma,
            "outbeta": out_beta, "out_b": out_b,
        })

    res = run_bass_kernel_spmd(nc, in_maps, core_ids=list(range(8)),
                               **_CACHE.get("run_kwargs", {}))
    if _CACHE.get("last_result_hook"):
        _CACHE["last_result_hook"](res)
    return np.concatenate([res.results[c]["out"] for c in range(8)], axis=0)


# revision 10
# speedup vs baseline: 1.1451x; 1.1451x over previous
"""EnergyTransformer Trainium2 kernel: 8-core data-parallel over batch.

Each core handles 2 images. State h kept d-major [768, 2*197] in SBUF fp32.
Matmuls in bf16 (1 cyc/row); layernorm stats via fp32r ones-matmuls.
"""
import numpy as np
import ml_dtypes
from contextlib import ExitStack

import concourse.bass as bass
import concourse.tile as tile
from concourse import bacc, mybir
from concourse.bass_utils import run_bass_kernel_spmd

BF16 = mybir.dt.bfloat16
F32 = mybir.dt.float32
F32R = mybir.dt.float32r
AF = mybir.ActivationFunctionType

D = 768            # embed dim
NT = 197           # tokens per image (incl cls)
NI = 2             # images per core
T = NI * NT        # 394 token columns
HOP = 3072
NHEAD = 12
HDIM = 64
NSTEP = 6
BETA = 1.0 / 8.0
EPS = 1e-5
P = 128
DC = D // P        # 6 d-chunks
HC = HOP // P      # 24 hopfield chunks

# token chunks: per image [128, 69]
TOK_CHUNKS = [(im, mc, im * NT + mc * P, 128 if mc == 0 else NT - P)
              for im in range(NI) for mc in range(2)]  # (img, mc, col0, valid)

_CACHE = {}


def _patch_act_tables():
    import concourse.bacc as _bacc
    import concourse.hw_specs as _hw
    if getattr(_bacc, "_act_tables_patched", False):
        return
    orig = _hw.get_activation_tables

    def patched(arch):
        tabs = orig(arch)
        keep = "natural_log_exp_and_others"
        if keep in tabs:
            strip = tabs[keep]
            out = {}
            for name, fns in tabs.items():
                if name == keep:
                    out[name] = fns
                else:
                    out[name] = fns - strip
            return out
        return tabs

    _bacc.get_activation_tables = patched
    _bacc._act_tables_patched = True


def _build():
    _patch_act_tables()
    nc = bacc.Bacc("TRN2", target_bir_lowering=False, debug=False, num_devices=8)

    # ---- DRAM params (per-core) ----
    patches = nc.declare_dram_parameter("patches", [D, 512], F32R, isOutput=False)
    h0T = nc.declare_dram_parameter("h0T", [D, T], F32, isOutput=False)
    pw = nc.declare_dram_parameter("pw", [D, D], F32R, isOutput=False)
    wq_d = nc.declare_dram_parameter("wq_d", [D, D], BF16, isOutput=False)
    wk_d = nc.declare_dram_parameter("wk_d", [D, D], BF16, isOutput=False)
    w_rot = nc.declare_dram_parameter("w_rot", [2 * D, D], BF16, isOutput=False)
    xi_d = nc.declare_dram_parameter("xi_d", [D, HOP], BF16, isOutput=False)
    xi_T = nc.declare_dram_parameter("xi_T", [HOP, D], BF16, isOutput=False)
    out_w = nc.declare_dram_parameter("out_w", [D, D], BF16, isOutput=False)
    lng = nc.declare_dram_parameter("lng", [D], F32, isOutput=False)
    lnb = nc.declare_dram_parameter("lnb", [D], F32, isOutput=False)
    outg = nc.declare_dram_parameter("outg", [D], F32, isOutput=False)
    outbeta = nc.declare_dram_parameter("outbeta", [D], F32, isOutput=False)
    out_b = nc.declare_dram_parameter("out_b", [D], F32, isOutput=False)
    out = nc.declare_dram_parameter("out", [NI, NT - 1, D], F32, isOutput=True)

    with tile.TileContext(nc) as tc, ExitStack() as ctx:
        const = ctx.enter_context(tc.tile_pool(name="const", bufs=1))
        state = ctx.enter_context(tc.tile_pool(name="state", bufs=1))
        bigw = ctx.enter_context(tc.tile_pool(name="bigw", bufs=1))
        p_patch = ctx.enter_context(tc.tile_pool(name="p_patch", bufs=6))
        p_sq = ctx.enter_context(tc.tile_pool(name="p_sq", bufs=6))
        p_hr = ctx.enter_context(tc.tile_pool(name="p_hr", bufs=6))
        p_row = ctx.enter_context(tc.tile_pool(name="p_row", bufs=4))
        p_R = ctx.enter_context(tc.tile_pool(name="p_R", bufs=8))
        p_RT = ctx.enter_context(tc.tile_pool(name="p_RT", bufs=5))
        p_st = ctx.enter_context(tc.tile_pool(name="p_st", bufs=5))
        p_ln = ctx.enter_context(tc.tile_pool(name="p_ln", bufs=5))
        p_hid = ctx.enter_context(tc.tile_pool(name="p_hid", bufs=3))
        p_xiT = ctx.enter_context(tc.tile_pool(name="p_xiT", bufs=6))
        p_out = ctx.enter_context(tc.tile_pool(name="p_out", bufs=2))
        ps = ctx.enter_context(tc.tile_pool(name="ps", bufs=8, space="PSUM"))

        def psum(pp=P, ff=512):
            t = ps.tile([pp, ff], F32, tag="ps", name="pst")
            return t[:pp, :ff]

        # ---- constants / weights resident ----
        wq_t = const.tile([P, DC, D], BF16)
        nc.sync.dma_start(wq_t[:], wq_d.rearrange("(o p) c -> p o c", p=P))
        wk_t = const.tile([P, DC, D], BF16)
        nc.sync.dma_start(wk_t[:], wk_d.rearrange("(o p) c -> p o c", p=P))
        wrot_t = const.tile([P, 2 * DC, D], BF16)
        nc.sync.dma_start(wrot_t[:], w_rot.rearrange("(o p) d -> p o d", p=P))
        xi_t = const.tile([P, DC, HOP], BF16)
        nc.sync.dma_start(xi_t[:], xi_d.rearrange("(o p) j -> p o j", p=P))
        lng_t = const.tile([P, DC], F32)
        nc.sync.dma_start(lng_t[:], lng.rearrange("(o p) -> p o", p=P))
        lnb_t = const.tile([P, DC], F32)
        nc.sync.dma_start(lnb_t[:], lnb.rearrange("(o p) -> p o", p=P))
        outg_t = const.tile([P, DC], F32)
        nc.sync.dma_start(outg_t[:], outg.rearrange("(o p) -> p o", p=P))
        outbeta_t = const.tile([P, DC], F32)
        nc.sync.dma_start(outbeta_t[:], outbeta.rearrange("(o p) -> p o", p=P))
        outb_row = const.tile([1, D], F32)
        nc.sync.dma_start(outb_row[:], out_b[None, :])

        ones_colf = const.tile([P, 1], F32)
        nc.vector.memset(ones_colf[:], 1.0)
        ones_row = const.tile([1, P], F32)
        nc.vector.memset(ones_row[:], 1.0)
        neg8_row = const.tile([1, P], BF16)
        nc.vector.memset(neg8_row[:], -8.0)
        ident = const.tile([P, P], F32)
        from concourse.masks import make_identity
        make_identity(nc, ident[:])
        eps_t = const.tile([1, 1], F32)
        nc.vector.memset(eps_t[:], EPS)

        # ---- state ----
        HT = state.tile([P, DC, T], F32)       # h d-major
        gT = state.tile([P, DC, T], BF16)      # LN(h) d-major
        QT = state.tile([P, DC, T], BF16)      # q^T (head-major rows)
        KT = state.tile([P, DC, T], BF16)
        Qtok = state.tile([P, 4, D], BF16)     # q token-major (4 tok chunks)
        Ktok = state.tile([P, 4, D], BF16)
        UVT = state.tile([P, 2 * DC, T], BF16)  # [U^T; V^T]

        # ---- pre-stage: h0 = bias ; h0 += patches @ patch_w ----
        nc.sync.dma_start(HT[:], h0T.rearrange("(o p) t -> p o t", p=P))
        pw_t = bigw.tile([P, DC, D], F32R, tag="bigw")
        nc.sync.dma_start(pw_t[:], pw.rearrange("(o p) d -> p o d", p=P))
        for im in range(NI):
            pch = []
            for k in range(DC):
                t = p_patch.tile([P, 256], F32R, tag="patch")
                nc.sync.dma_start(t[:], patches[k * P:(k + 1) * P,
                                               im * 256:(im + 1) * 256])
                pch.append(t)
            for i in range(DC):
                pp = psum(P, 256)
                for k in range(DC):
                    nc.tensor.matmul(pp, pw_t[:, k, i * P:(i + 1) * P],
                                     pch[k][:],
                                     start=(k == 0), stop=(k == DC - 1))
                cc = im * NT + 1
                nc.vector.tensor_add(HT[:, i, cc:cc + NT - 1], HT[:, i, cc:cc + NT - 1],
                                     pp[:, 0:NT - 1])

        # ---- helpers ----
        def layernorm(gamma_t, beta_t, out_tile):
            sqs = []
            for i in range(DC):
                sq = p_sq.tile([P, T], F32, tag="sq")
                nc.vector.tensor_mul(sq[:], HT[:, i, :], HT[:, i, :])
                sqs.append(sq)
            mu_ps = psum(1, T)
            for i in range(DC):
                nc.tensor.matmul(mu_ps, ones_colf[:], HT[:, i, :],
                                 start=(i == 0), stop=(i == DC - 1))
            ss_ps = psum(1, T)
            for i in range(DC):
                nc.tensor.matmul(ss_ps, ones_colf[:], sqs[i][:],
                                 start=(i == 0), stop=(i == DC - 1))
            t_mu = p_row.tile([1, T], F32, tag="row")
            nc.vector.tensor_scalar_mul(t_mu[:], mu_ps[0:1, :], 1.0 / D)
            t_var = p_row.tile([1, T], F32, tag="row")
            nc.vector.tensor_scalar_mul(t_var[:], ss_ps[0:1, :], 1.0 / D)
            t_ms = p_row.tile([1, T], F32, tag="row")
            nc.vector.tensor_mul(t_ms[:], t_mu[:], t_mu[:])
            nc.vector.tensor_sub(t_var[:], t_var[:], t_ms[:])
            nc.scalar.activation(t_var[:], t_var[:], AF.Ln, bias=eps_t[0:1, :])
            a_row = p_row.tile([1, T], F32, tag="row")
            nc.scalar.activation(a_row[:], t_var[:], AF.Exp, scale=-0.5)
            b_row = t_ms
            nc.vector.tensor_mul(b_row[:], t_mu[:], a_row[:])
            nc.vector.tensor_scalar_mul(b_row[:], b_row[:], -1.0)
            a_ps = psum(P, T)
            nc.tensor.matmul(a_ps, ones_row[0:1, :], a_row[0:1, :],
                             start=True, stop=True)
            b_ps = psum(P, T)
            nc.tensor.matmul(b_ps, ones_row[0:1, :], b_row[0:1, :],
                             start=True, stop=True)
            for i in range(DC):
                t = p_hr.tile([P, T], F32, tag="gtmp", name="gtmp")
                nc.vector.tensor_tensor(t[:], HT[:, i, :], a_ps[:, :],
                                        mybir.AluOpType.mult)
                nc.vector.tensor_tensor(t[:], t[:], b_ps[:, :],
                                        mybir.AluOpType.add)
                nc.vector.tensor_scalar(out_tile[:, i, :], t[:],
                                        gamma_t[:, i:i + 1], beta_t[:, i:i + 1],
                                        mybir.AluOpType.mult, mybir.AluOpType.add)

        # ---- 6 time steps ----
        for step in range(NSTEP):
            layernorm(lng_t, lnb_t, gT)

            # QT/KT d-major: for chunk i, rows = heads (2i, 2i+1)
            for (w_t, dst) in ((wq_t, QT), (wk_t, KT)):
                for i in range(DC):
                    pp = psum(P, T)
                    for k in range(DC):
                        nc.tensor.matmul(pp, w_t[:, k, i * P:(i + 1) * P],
                                         gT[:, k, :],
                                         start=(k == 0), stop=(k == DC - 1))
                    nc.vector.tensor_copy(dst[:, i, :], pp[:, :])

            # Qtok/Ktok token-major
            for (w_t, dst) in ((wq_t, Qtok), (wk_t, Ktok)):
                for tci, (im, mc, c0, vm) in enumerate(TOK_CHUNKS):
                    for b in range(2):
                        pp = psum(vm, 384)
                        for k in range(DC):
                            nc.tensor.matmul(pp, gT[:, k, c0:c0 + vm],
                                             w_t[:, k, b * 384:(b + 1) * 384],
                                             start=(k == 0), stop=(k == DC - 1))
                        nc.vector.tensor_copy(dst[:vm, tci, b * 384:(b + 1) * 384], pp)

            # attention: 12 (img, head-pair) units, software-pipelined lag 1
            pairs = [(im, hp) for im in range(NI) for hp in range(DC)]
            saved = {}

            def emit_A(pidx):
                im, hp = pairs[pidx]
                ic0 = im * NT
                Rt, st2 = {}, {}
                for mc in range(2):
                    c0 = ic0 + mc * P
                    vm = 128 if mc == 0 else NT - P
                    st2[mc] = p_st.tile([P, 2], F32, tag="st2", name="st2")
                    for hh in range(2):
                        off = hh * HDIM
                        pa = psum(vm, NT)
                        nc.tensor.matmul(pa, QT[off:off + HDIM, hp, c0:c0 + vm],
                                         KT[off:off + HDIM, hp, ic0:ic0 + NT],
                                         start=True, stop=True)
                        E = p_R.tile([P, NT], BF16, tag="R")
                        nc.scalar.activation(E[:vm, :], pa, AF.Exp, scale=BETA,
                                             accum_out=st2[mc][:vm, hh:hh + 1])
                        Rt[(hh, mc)] = E
                    rd = p_st.tile([P, 2], F32, tag="rd")
                    nc.vector.reciprocal(rd[:vm, :], st2[mc][:vm, :])
                    for hh in range(2):
                        nc.vector.tensor_scalar_mul(Rt[(hh, mc)][:vm, :],
                                                    Rt[(hh, mc)][:vm, :],
                                                    rd[:vm, hh:hh + 1])
                saved[pidx] = (Rt, st2)

            def emit_rest(pidx):
                im, hp = pairs[pidx]
                ic0 = im * NT
                Rt, st2 = saved.pop(pidx)
                # stats transpose -> ln rows
                lnr = [p_ln.tile([1, NT], BF16, tag="lnr", name="lnr") for _ in range(2)]
                for mc in range(2):
                    vm = 128 if mc == 0 else NT - P
                    for hh in range(2):
                        pt = psum(1, P)
                        nc.tensor.transpose(pt[:1, :vm], st2[mc][:vm, hh:hh + 1],
                                            ident[:vm, :vm])
                        nc.scalar.activation(lnr[hh][0:1, mc * P:mc * P + vm],
                                             pt[0:1, :vm], AF.Ln,
                                             scale=1.0 / NT)
                # V^T = sum_mc Qtok_slice.T @ R
                for hh in range(2):
                    off = hh * HDIM
                    pv = psum(HDIM, NT)
                    for mc in range(2):
                        vm = 128 if mc == 0 else NT - P
                        tci = im * 2 + mc
                        nc.tensor.matmul(pv, Qtok[:vm, tci, hp * P + off:hp * P + off + HDIM],
                                         Rt[(hh, mc)][:vm, :],
                                         start=(mc == 0), stop=(mc == 1))
                    nc.vector.tensor_copy(UVT[off:off + HDIM, DC + hp, ic0:ic0 + NT], pv)
                # B scores + rank1(-8*lnsum) -> exp -> R^T ; U^T
                for hh in range(2):
                    off = hh * HDIM
                    RTs = []
                    for mc in range(2):
                        vn = 128 if mc == 0 else NT - P
                        c0 = ic0 + mc * P
                        pb = psum(vn, NT)
                        nc.tensor.matmul(pb, KT[off:off + HDIM, hp, c0:c0 + vn],
                                         QT[off:off + HDIM, hp, ic0:ic0 + NT],
                                         start=True, stop=False)
                        nc.tensor.matmul(pb, neg8_row[0:1, 0:vn], lnr[hh][0:1, :],
                                         start=False, stop=True)
                        RTt = p_RT.tile([P, NT], BF16, tag="RT")
                        nc.scalar.activation(RTt[:vn, :], pb, AF.Exp, scale=BETA)
                        RTs.append((RTt, vn))
                    pu = psum(HDIM, NT)
                    for mc in range(2):
                        RTt, vn = RTs[mc]
                        tci = im * 2 + mc
                        nc.tensor.matmul(pu, Ktok[:vn, tci, hp * P + off:hp * P + off + HDIM],
                                         RTt[:vn, :],
                                         start=(mc == 0), stop=(mc == 1))
                    nc.vector.tensor_scalar_mul(UVT[off:off + HDIM, hp, ic0:ic0 + NT],
                                                pu, 1.0 / NT)

            for idx in range(len(pairs) + 1):
                if idx < len(pairs):
                    emit_A(idx)
                if idx >= 1:
                    emit_rest(idx - 1)

            # delta_attn^T accumulate (6 banks held)
            dps = []
            for i in range(DC):
                pp = psum(P, T)
                for j in range(2 * DC):
                    nc.tensor.matmul(pp, wrot_t[:, j, i * P:(i + 1) * P],
                                     UVT[:, j, :],
                                     start=(j == 0), stop=(j == 2 * DC - 1))
                dps.append(pp)
            for i in range(DC):
                nc.vector.tensor_add(HT[:, i, :], HT[:, i, :], dps[i][:, :])

            # hopfield fwd+bwd fused; 6 accumulators
            hps = [psum(P, T) for _ in range(DC)]
            for c in range(HC):
                xiTt = p_xiT.tile([P, D], BF16, tag="xiT")
                nc.sync.dma_start(xiTt[:],
                                  xi_T.rearrange("(o p) d -> p o d", p=P)[:, c, :])
                pf = psum(P, T)
                for k in range(DC):
                    nc.tensor.matmul(pf, xi_t[:, k, c * P:(c + 1) * P],
                                     gT[:, k, :],
                                     start=(k == 0), stop=(k == DC - 1))
                hid = p_hid.tile([P, T], BF16, tag="hid")
                nc.scalar.activation(hid[:], pf, AF.Relu)
                for i in range(DC):
                    nc.tensor.matmul(hps[i], xiTt[:, i * P:(i + 1) * P], hid[:],
                                     start=(c == 0), stop=(c == HC - 1))
            for i in range(DC):
                nc.vector.tensor_add(HT[:, i, :], HT[:, i, :], hps[i][:, :])

        # ---- output stage ----
        layernorm(outg_t, outbeta_t, gT)
        ow_t = bigw.tile([P, DC, D], BF16, tag="bigw")
        nc.sync.dma_start(ow_t[:], out_w.rearrange("(o p) d -> p o d", p=P))
        obb = const.tile([P, D], F32, tag="obb", name="obb")
        for b in range(2):
            pp = psum(P, 384)
            nc.tensor.matmul(pp, ones_row[0:1, :], outb_row[0:1, b * 384:(b + 1) * 384],
                             start=True, stop=True)
            nc.vector.tensor_copy(obb[:, b * 384:(b + 1) * 384], pp)
        for tci, (im, mc, c0, vm) in enumerate(TOK_CHUNKS):
            osb = p_out.tile([P, D], F32, tag="osb")
            for b in range(2):
                pp = psum(vm, 384)
                for k in range(DC):
                    nc.tensor.matmul(pp, gT[:, k, c0:c0 + vm],
                                     ow_t[:, k, b * 384:(b + 1) * 384],
                                     start=(k == 0), stop=(k == DC - 1))
                nc.vector.tensor_tensor(osb[:vm, b * 384:(b + 1) * 384], pp,
                                        obb[:vm, b * 384:(b + 1) * 384],
                                        mybir.AluOpType.add)
            if mc == 0:
                nc.sync.dma_start(out[im, 0:P - 1, :], osb[1:P, :])
            else:
                nc.sync.dma_start(out[im, P - 1:NT - 1, :], osb[0:vm, :])

    nc.finalize()
    return nc


def _get_nc():
    if "nc" not in _CACHE:
        _CACHE["nc"] = _build()
    return _CACHE["nc"]


def kernel(**inputs):
    x = np.asarray(inputs["x"], dtype=np.float32)            # [16,3,224,224]
    patch_w = np.asarray(inputs["patch_w"], dtype=np.float32)
    patch_b = np.asarray(inputs["patch_b"], dtype=np.float32)
    cls_token = np.asarray(inputs["cls_token"], dtype=np.float32)
    pos_embed = np.asarray(inputs["pos_embed"], dtype=np.float32)
    ln_gamma = np.asarray(inputs["ln_gamma"], dtype=np.float32)
    ln_beta = np.asarray(inputs["ln_beta"], dtype=np.float32)
    wq = np.asarray(inputs["wq"], dtype=np.float32)
    wk = np.asarray(inputs["wk"], dtype=np.float32)
    xi = np.asarray(inputs["xi"], dtype=np.float32)
    out_gamma = np.asarray(inputs["out_gamma"], dtype=np.float32)
    out_beta = np.asarray(inputs["out_beta"], dtype=np.float32)
    out_w = np.asarray(inputs["out_w"], dtype=np.float32)
    out_b = np.asarray(inputs["out_b"], dtype=np.float32)

    B = x.shape[0]
    bf = ml_dtypes.bfloat16
    # host-side prep (shared across cores)
    wq_d = np.ascontiguousarray(wq.transpose(1, 0, 2).reshape(D, D)).astype(bf)
    wk_d = np.ascontiguousarray(wk.transpose(1, 0, 2).reshape(D, D)).astype(bf)
    w_rot = np.concatenate([wq.transpose(0, 2, 1).reshape(D, D),
                            wk.transpose(0, 2, 1).reshape(D, D)], axis=0).astype(bf)
    xi_b = xi.astype(bf)
    xi_Tb = np.ascontiguousarray(xi.T).astype(bf)
    out_wb = out_w.astype(bf)

    # h0 bias columns: [768, 394]
    pos = pos_embed[0]                                        # [197, 768]
    h0 = np.empty((NT, D), dtype=np.float32)
    h0[0] = cls_token[0, 0] + pos[0]
    h0[1:] = pos[1:] + patch_b[None, :]
    h0T_one = np.ascontiguousarray(h0.T)                      # [768, 197]
    h0T = np.concatenate([h0T_one, h0T_one], axis=1)          # [768, 394]

    # patchify (host): [B, 196, 768]
    xr = x.reshape(B, 3, 14, 16, 14, 16).transpose(0, 2, 4, 1, 3, 5).reshape(B, 196, D)

    def tf32_round(a):
        b = a.astype(np.float32).copy()
        b.view(np.uint32)[...] &= np.uint32(0xFFFFE000)
        return b

    patch_w_r = tf32_round(patch_w)
    nc = _get_nc()
    in_maps = []
    for c in range(8):
        pT = np.zeros((D, 512), dtype=np.float32)
        pT[:, 0:196] = xr[2 * c].T
        pT[:, 256:452] = xr[2 * c + 1].T
        in_maps.append({
            "patches": tf32_round(pT), "h0T": h0T, "pw": patch_w_r,
            "wq_d": wq_d, "wk_d": wk_d, "w_rot": w_rot,
            "xi_d": xi_b, "xi_T": xi_Tb, "out_w": out_wb,
            "lng": ln_gamma, "lnb": ln_beta, "outg": out_gamma,
            "outbeta": out_beta, "out_b": out_b,
        })

    res = run_bass_kernel_spmd(nc, in_maps, core_ids=list(range(8)),
                               **_CACHE.get("run_kwargs", {}))
    if _CACHE.get("last_result_hook"):
        _CACHE["last_result_hook"](res)
    return np.concatenate([res.results[c]["out"] for c in range(8)], axis=0)


# revision 13
# speedup vs baseline: 1.2738x; 1.1124x over previous
"""EnergyTransformer Trainium2 kernel: 8-core data-parallel over batch.

Each core handles 2 images. State h kept d-major [768, 2*197] in SBUF fp32.
Matmuls in bf16 (1 cyc/row); layernorm stats via fp32r ones-matmuls.
"""
import numpy as np
import ml_dtypes
from contextlib import ExitStack

import concourse.bass as bass
import concourse.tile as tile
from concourse import bacc, mybir
from concourse.bass_utils import run_bass_kernel_spmd

BF16 = mybir.dt.bfloat16
F32 = mybir.dt.float32
F32R = mybir.dt.float32r
AF = mybir.ActivationFunctionType

D = 768            # embed dim
NT = 197           # tokens per image (incl cls)
NI = 2             # images per core
T = NI * NT        # 394 token columns
HOP = 3072
NHEAD = 12
HDIM = 64
NSTEP = 6
BETA = 1.0 / 8.0
EPS = 1e-5
P = 128
DC = D // P        # 6 d-chunks
HC = HOP // P      # 24 hopfield chunks

# token chunks: per image [128, 69]
TOK_CHUNKS = [(im, mc, im * NT + mc * P, 128 if mc == 0 else NT - P)
              for im in range(NI) for mc in range(2)]  # (img, mc, col0, valid)

_CACHE = {}


def _patch_act_tables():
    import concourse.bacc as _bacc
    import concourse.hw_specs as _hw
    if getattr(_bacc, "_act_tables_patched", False):
        return
    orig = _hw.get_activation_tables

    def patched(arch):
        tabs = orig(arch)
        keep = "natural_log_exp_and_others"
        if keep in tabs:
            strip = tabs[keep]
            out = {}
            for name, fns in tabs.items():
                if name == keep:
                    out[name] = fns
                else:
                    out[name] = fns - strip
            return out
        return tabs

    _bacc.get_activation_tables = patched
    _bacc._act_tables_patched = True


def _build():
    _patch_act_tables()
    nc = bacc.Bacc("TRN2", target_bir_lowering=False, debug=False, num_devices=8)

    # ---- DRAM params (per-core) ----
    patches = nc.declare_dram_parameter("patches", [D, 512], F32R, isOutput=False)
    h0T = nc.declare_dram_parameter("h0T", [D, T], F32, isOutput=False)
    pw = nc.declare_dram_parameter("pw", [D, D], F32R, isOutput=False)
    wq_d = nc.declare_dram_parameter("wq_d", [D, D], BF16, isOutput=False)
    wk_d = nc.declare_dram_parameter("wk_d", [D, D], BF16, isOutput=False)
    w_rot = nc.declare_dram_parameter("w_rot", [2 * D, D], BF16, isOutput=False)
    xi_d = nc.declare_dram_parameter("xi_d", [D, HOP], BF16, isOutput=False)
    xi_T = nc.declare_dram_parameter("xi_T", [HOP, D], BF16, isOutput=False)
    out_w = nc.declare_dram_parameter("out_w", [D, D], BF16, isOutput=False)
    lng = nc.declare_dram_parameter("lng", [D], F32, isOutput=False)
    lnb = nc.declare_dram_parameter("lnb", [D], F32, isOutput=False)
    outg = nc.declare_dram_parameter("outg", [D], F32, isOutput=False)
    outbeta = nc.declare_dram_parameter("outbeta", [D], F32, isOutput=False)
    out_b = nc.declare_dram_parameter("out_b", [D], F32, isOutput=False)
    out = nc.declare_dram_parameter("out", [NI, NT - 1, D], F32, isOutput=True)

    with tile.TileContext(nc) as tc, ExitStack() as ctx:
        const = ctx.enter_context(tc.tile_pool(name="const", bufs=1))
        state = ctx.enter_context(tc.tile_pool(name="state", bufs=1))
        bigw = ctx.enter_context(tc.tile_pool(name="bigw", bufs=1))
        p_patch = ctx.enter_context(tc.tile_pool(name="p_patch", bufs=6))
        p_sq = ctx.enter_context(tc.tile_pool(name="p_sq", bufs=6))
        p_hr = ctx.enter_context(tc.tile_pool(name="p_hr", bufs=6))
        p_row = ctx.enter_context(tc.tile_pool(name="p_row", bufs=4))
        p_R = ctx.enter_context(tc.tile_pool(name="p_R", bufs=8))
        p_RT = ctx.enter_context(tc.tile_pool(name="p_RT", bufs=5))
        p_st = ctx.enter_context(tc.tile_pool(name="p_st", bufs=5))
        p_hid = ctx.enter_context(tc.tile_pool(name="p_hid", bufs=3))
        p_xiT = ctx.enter_context(tc.tile_pool(name="p_xiT", bufs=6))
        p_out = ctx.enter_context(tc.tile_pool(name="p_out", bufs=2))
        ps = ctx.enter_context(tc.tile_pool(name="ps", bufs=8, space="PSUM"))

        def psum(pp=P, ff=512, dt=F32):
            t = ps.tile([pp, ff], dt, tag="ps", name="pst")
            return t[:pp, :ff]

        # ---- constants / weights resident ----
        wq_t = const.tile([P, DC, D], BF16)
        nc.sync.dma_start(wq_t[:], wq_d.rearrange("(o p) c -> p o c", p=P))
        wk_t = const.tile([P, DC, D], BF16)
        nc.sync.dma_start(wk_t[:], wk_d.rearrange("(o p) c -> p o c", p=P))
        wrot_t = const.tile([P, 2 * DC, D], BF16)
        nc.sync.dma_start(wrot_t[:], w_rot.rearrange("(o p) d -> p o d", p=P))
        xi_t = const.tile([P, DC, HOP], BF16)
        nc.sync.dma_start(xi_t[:], xi_d.rearrange("(o p) j -> p o j", p=P))
        lng_t = const.tile([P, DC], F32)
        nc.sync.dma_start(lng_t[:], lng.rearrange("(o p) -> p o", p=P))
        lnb_t = const.tile([P, DC], F32)
        nc.sync.dma_start(lnb_t[:], lnb.rearrange("(o p) -> p o", p=P))
        outg_t = const.tile([P, DC], F32)
        nc.sync.dma_start(outg_t[:], outg.rearrange("(o p) -> p o", p=P))
        outbeta_t = const.tile([P, DC], F32)
        nc.sync.dma_start(outbeta_t[:], outbeta.rearrange("(o p) -> p o", p=P))
        outb_row = const.tile([1, D], F32)
        nc.sync.dma_start(outb_row[:], out_b[None, :])

        ones_colf = const.tile([P, 1], F32)
        nc.vector.memset(ones_colf[:], 1.0)
        ones_colr = const.tile([P, 1], F32R)
        nc.vector.tensor_copy(ones_colr[:], ones_colf[:])
        ones_row = const.tile([1, P], F32)
        nc.vector.memset(ones_row[:], 1.0)
        ident = const.tile([P, P], F32)
        from concourse.masks import make_identity
        make_identity(nc, ident[:])
        identb = const.tile([P, P], BF16)
        nc.vector.tensor_copy(identb[:], ident[:])
        eps_t = const.tile([1, 1], F32)
        nc.vector.memset(eps_t[:], EPS)

        # ---- state ----
        HT = state.tile([P, DC, T], F32)       # h d-major
        gT = state.tile([P, DC, T], BF16)      # LN(h) d-major
        QT = state.tile([P, DC, T], BF16)      # q^T (head-major rows)
        KT = state.tile([P, DC, T], BF16)
        Qtok = state.tile([P, 4, D], BF16)     # q token-major (4 tok chunks)
        Ktok = state.tile([P, 4, D], BF16)
        UVT = state.tile([P, 2 * DC, T], BF16)  # [U^T; V^T]

        # ---- pre-stage: h0 = bias ; h0 += patches @ patch_w ----
        nc.sync.dma_start(HT[:], h0T.rearrange("(o p) t -> p o t", p=P))
        pw_t = bigw.tile([P, DC, D], F32R, tag="bigw")
        nc.sync.dma_start(pw_t[:], pw.rearrange("(o p) d -> p o d", p=P))
        for im in range(NI):
            pch = []
            for k in range(DC):
                t = p_patch.tile([P, 256], F32R, tag="patch")
                nc.sync.dma_start(t[:], patches[k * P:(k + 1) * P,
                                               im * 256:(im + 1) * 256])
                pch.append(t)
            for i in range(DC):
                pp = psum(P, 256)
                for k in range(DC):
                    nc.tensor.matmul(pp, pw_t[:, k, i * P:(i + 1) * P],
                                     pch[k][:],
                                     start=(k == 0), stop=(k == DC - 1))
                cc = im * NT + 1
                nc.vector.tensor_add(HT[:, i, cc:cc + NT - 1], HT[:, i, cc:cc + NT - 1],
                                     pp[:, 0:NT - 1])

        # ---- helpers ----
        def layernorm(gamma_t, beta_t, out_tile):
            sqs = []
            for i in range(DC):
                sq = p_sq.tile([P, T], F32R, tag="sq")
                nc.vector.tensor_mul(sq[:], HT[:, i, :], HT[:, i, :])
                sqs.append(sq)
            mu_ps = psum(1, T)
            for i in range(DC):
                nc.tensor.matmul(mu_ps, ones_colf[:], HT[:, i, :],
                                 start=(i == 0), stop=(i == DC - 1))
            ss_ps = psum(1, T)
            for i in range(DC):
                nc.tensor.matmul(ss_ps, ones_colr[:], sqs[i][:],
                                 start=(i == 0), stop=(i == DC - 1))
            t_mu = p_row.tile([1, T], F32, tag="row")
            nc.vector.tensor_scalar_mul(t_mu[:], mu_ps[0:1, :], 1.0 / D)
            t_var = p_row.tile([1, T], F32, tag="row")
            nc.vector.tensor_scalar_mul(t_var[:], ss_ps[0:1, :], 1.0 / D)
            t_ms = p_row.tile([1, T], F32, tag="row")
            nc.vector.tensor_mul(t_ms[:], t_mu[:], t_mu[:])
            nc.vector.tensor_sub(t_var[:], t_var[:], t_ms[:])
            nc.scalar.activation(t_var[:], t_var[:], AF.Ln, bias=eps_t[0:1, :])
            a_row = p_row.tile([1, T], F32, tag="row")
            nc.scalar.activation(a_row[:], t_var[:], AF.Exp, scale=-0.5)
            b_row = t_ms
            nc.vector.tensor_mul(b_row[:], t_mu[:], a_row[:])
            nc.vector.tensor_scalar_mul(b_row[:], b_row[:], -1.0)
            a_ps = psum(P, T)
            nc.tensor.matmul(a_ps, ones_row[0:1, :], a_row[0:1, :],
                             start=True, stop=True)
            b_ps = psum(P, T)
            nc.tensor.matmul(b_ps, ones_row[0:1, :], b_row[0:1, :],
                             start=True, stop=True)
            for i in range(DC):
                t = p_hr.tile([P, T], F32, tag="gtmp", name="gtmp")
                nc.vector.tensor_tensor(t[:], HT[:, i, :], a_ps[:, :],
                                        mybir.AluOpType.mult)
                nc.vector.tensor_tensor(t[:], t[:], b_ps[:, :],
                                        mybir.AluOpType.add)
                nc.vector.tensor_scalar(out_tile[:, i, :], t[:],
                                        gamma_t[:, i:i + 1], beta_t[:, i:i + 1],
                                        mybir.AluOpType.mult, mybir.AluOpType.add)

        # ---- 6 time steps ----
        for step in range(NSTEP):
            layernorm(lng_t, lnb_t, gT)

            # QT/KT d-major: for chunk i, rows = heads (2i, 2i+1)
            for (w_t, dst) in ((wq_t, QT), (wk_t, KT)):
                for i in range(DC):
                    pp = psum(P, T)
                    for k in range(DC):
                        nc.tensor.matmul(pp, w_t[:, k, i * P:(i + 1) * P],
                                         gT[:, k, :],
                                         start=(k == 0), stop=(k == DC - 1))
                    nc.vector.tensor_copy(dst[:, i, :], pp[:, :])

            # Qtok/Ktok token-major
            for (w_t, dst) in ((wq_t, Qtok), (wk_t, Ktok)):
                for tci, (im, mc, c0, vm) in enumerate(TOK_CHUNKS):
                    for b in range(2):
                        pp = psum(vm, 384)
                        for k in range(DC):
                            nc.tensor.matmul(pp, gT[:, k, c0:c0 + vm],
                                             w_t[:, k, b * 384:(b + 1) * 384],
                                             start=(k == 0), stop=(k == DC - 1))
                        nc.vector.tensor_copy(dst[:vm, tci, b * 384:(b + 1) * 384], pp)

            # attention: 12 (img, head-pair) units, software-pipelined lag 1
            pairs = [(im, hp) for im in range(NI) for hp in range(DC)]
            saved = {}

            def emit_A(pidx):
                im, hp = pairs[pidx]
                ic0 = im * NT
                Rt, st2 = {}, {}
                for mc in range(2):
                    c0 = ic0 + mc * P
                    vm = 128 if mc == 0 else NT - P
                    st2[mc] = p_st.tile([P, 2], F32, tag="st2", name="st2")
                    for hh in range(2):
                        off = hh * HDIM
                        pa = psum(vm, NT)
                        nc.tensor.matmul(pa, QT[off:off + HDIM, hp, c0:c0 + vm],
                                         KT[off:off + HDIM, hp, ic0:ic0 + NT],
                                         start=True, stop=True)
                        E = p_R.tile([P, NT], BF16, tag="R")
                        nc.scalar.activation(E[:vm, :], pa, AF.Exp, scale=BETA,
                                             accum_out=st2[mc][:vm, hh:hh + 1])
                        Rt[(hh, mc)] = E
                    rd = p_st.tile([P, 2], F32, tag="rd")
                    nc.vector.reciprocal(rd[:vm, :], st2[mc][:vm, :])
                    for hh in range(2):
                        nc.vector.tensor_scalar_mul(Rt[(hh, mc)][:vm, :],
                                                    Rt[(hh, mc)][:vm, :],
                                                    rd[:vm, hh:hh + 1])
                saved[pidx] = (Rt, st2)

            def emit_rest(pidx):
                im, hp = pairs[pidx]
                ic0 = im * NT
                Rt, st2 = saved.pop(pidx)
                # V^T = sum_mc Qtok_slice.T @ R
                for hh in range(2):
                    off = hh * HDIM
                    pv = psum(HDIM, NT)
                    for mc in range(2):
                        vm = 128 if mc == 0 else NT - P
                        tci = im * 2 + mc
                        nc.tensor.matmul(pv, Qtok[:vm, tci, hp * P + off:hp * P + off + HDIM],
                                         Rt[(hh, mc)][:vm, :],
                                         start=(mc == 0), stop=(mc == 1))
                    nc.vector.tensor_copy(UVT[off:off + HDIM, DC + hp, ic0:ic0 + NT], pv)
                # R^T via PE transposes of R blocks; U^T = sum_nc Ktok_slice.T @ R^T
                for hh in range(2):
                    off = hh * HDIM
                    RTs = []
                    for nc_ in range(2):
                        vn = 128 if nc_ == 0 else NT - P
                        RTt = p_RT.tile([P, NT], BF16, tag="RT", name="RTt")
                        for mc in range(2):
                            vm = 128 if mc == 0 else NT - P
                            ptp = psum(P, P, BF16)
                            nc.tensor.transpose(ptp[:vn, :vm],
                                                Rt[(hh, mc)][:vm, nc_ * P:nc_ * P + vn],
                                                identb[:vm, :vm])
                            nc.scalar.copy(RTt[:vn, mc * P:mc * P + vm],
                                           ptp[:vn, :vm])
                        RTs.append((RTt, vn))
                    pu = psum(HDIM, NT)
                    for nc_ in range(2):
                        RTt, vn = RTs[nc_]
                        tci = im * 2 + nc_
                        nc.tensor.matmul(pu, Ktok[:vn, tci, hp * P + off:hp * P + off + HDIM],
                                         RTt[:vn, :],
                                         start=(nc_ == 0), stop=(nc_ == 1))
                    nc.vector.tensor_copy(UVT[off:off + HDIM, hp, ic0:ic0 + NT], pu)

            for idx in range(len(pairs) + 1):
                if idx < len(pairs):
                    emit_A(idx)
                if idx >= 1:
                    emit_rest(idx - 1)

            # delta_attn^T accumulate (6 banks held)
            dps = []
            for i in range(DC):
                pp = psum(P, T)
                for j in range(2 * DC):
                    nc.tensor.matmul(pp, wrot_t[:, j, i * P:(i + 1) * P],
                                     UVT[:, j, :],
                                     start=(j == 0), stop=(j == 2 * DC - 1))
                dps.append(pp)
            for i in range(DC):
                nc.vector.tensor_add(HT[:, i, :], HT[:, i, :], dps[i][:, :])

            # hopfield fwd+bwd fused; 6 accumulators
            hps = [psum(P, T) for _ in range(DC)]
            for c in range(HC):
                xiTt = p_xiT.tile([P, D], BF16, tag="xiT")
                nc.sync.dma_start(xiTt[:],
                                  xi_T.rearrange("(o p) d -> p o d", p=P)[:, c, :])
                pf = psum(P, T)
                for k in range(DC):
                    nc.tensor.matmul(pf, xi_t[:, k, c * P:(c + 1) * P],
                                     gT[:, k, :],
                                     start=(k == 0), stop=(k == DC - 1))
                hid = p_hid.tile([P, T], BF16, tag="hid")
                nc.scalar.activation(hid[:], pf, AF.Relu)
                for i in range(DC):
                    nc.tensor.matmul(hps[i], xiTt[:, i * P:(i + 1) * P], hid[:],
                                     start=(c == 0), stop=(c == HC - 1))
            for i in range(DC):
                nc.vector.tensor_add(HT[:, i, :], HT[:, i, :], hps[i][:, :])

        # ---- output stage ----
        layernorm(outg_t, outbeta_t, gT)
        ow_t = bigw.tile([P, DC, D], BF16, tag="bigw")
        nc.sync.dma_start(ow_t[:], out_w.rearrange("(o p) d -> p o d", p=P))
        obb = const.tile([P, D], F32, tag="obb", name="obb")
        for b in range(2):
            pp = psum(P, 384)
            nc.tensor.matmul(pp, ones_row[0:1, :], outb_row[0:1, b * 384:(b + 1) * 384],
                             start=True, stop=True)
            nc.vector.tensor_copy(obb[:, b * 384:(b + 1) * 384], pp)
        for tci, (im, mc, c0, vm) in enumerate(TOK_CHUNKS):
            osb = p_out.tile([P, D], F32, tag="osb")
            for b in range(2):
                pp = psum(vm, 384)
                for k in range(DC):
                    nc.tensor.matmul(pp, gT[:, k, c0:c0 + vm],
                                     ow_t[:, k, b * 384:(b + 1) * 384],
                                     start=(k == 0), stop=(k == DC - 1))
                nc.vector.tensor_tensor(osb[:vm, b * 384:(b + 1) * 384], pp,
                                        obb[:vm, b * 384:(b + 1) * 384],
                                        mybir.AluOpType.add)
            if mc == 0:
                nc.sync.dma_start(out[im, 0:P - 1, :], osb[1:P, :])
            else:
                nc.sync.dma_start(out[im, P - 1:NT - 1, :], osb[0:vm, :])

    nc.finalize()
    return nc


def _get_nc():
    if "nc" not in _CACHE:
        _CACHE["nc"] = _build()
    return _CACHE["nc"]


def kernel(**inputs):
    x = np.asarray(inputs["x"], dtype=np.float32)            # [16,3,224,224]
    patch_w = np.asarray(inputs["patch_w"], dtype=np.float32)
    patch_b = np.asarray(inputs["patch_b"], dtype=np.float32)
    cls_token = np.asarray(inputs["cls_token"], dtype=np.float32)
    pos_embed = np.asarray(inputs["pos_embed"], dtype=np.float32)
    ln_gamma = np.asarray(inputs["ln_gamma"], dtype=np.float32)
    ln_beta = np.asarray(inputs["ln_beta"], dtype=np.float32)
    wq = np.asarray(inputs["wq"], dtype=np.float32)
    wk = np.asarray(inputs["wk"], dtype=np.float32)
    xi = np.asarray(inputs["xi"], dtype=np.float32)
    out_gamma = np.asarray(inputs["out_gamma"], dtype=np.float32)
    out_beta = np.asarray(inputs["out_beta"], dtype=np.float32)
    out_w = np.asarray(inputs["out_w"], dtype=np.float32)
    out_b = np.asarray(inputs["out_b"], dtype=np.float32)

    B = x.shape[0]
    bf = ml_dtypes.bfloat16
    # host-side prep (shared across cores)
    wq_d = np.ascontiguousarray(wq.transpose(1, 0, 2).reshape(D, D)).astype(bf)
    wk_d = np.ascontiguousarray(wk.transpose(1, 0, 2).reshape(D, D)).astype(bf)
    w_rot = np.concatenate([wq.transpose(0, 2, 1).reshape(D, D),
                            wk.transpose(0, 2, 1).reshape(D, D)], axis=0).astype(bf)
    xi_b = xi.astype(bf)
    xi_Tb = np.ascontiguousarray(xi.T).astype(bf)
    out_wb = out_w.astype(bf)

    # h0 bias columns: [768, 394]
    pos = pos_embed[0]                                        # [197, 768]
    h0 = np.empty((NT, D), dtype=np.float32)
    h0[0] = cls_token[0, 0] + pos[0]
    h0[1:] = pos[1:] + patch_b[None, :]
    h0T_one = np.ascontiguousarray(h0.T)                      # [768, 197]
    h0T = np.concatenate([h0T_one, h0T_one], axis=1)          # [768, 394]

    # patchify (host): [B, 196, 768]
    xr = x.reshape(B, 3, 14, 16, 14, 16).transpose(0, 2, 4, 1, 3, 5).reshape(B, 196, D)

    def tf32_round(a):
        b = a.astype(np.float32).copy()
        b.view(np.uint32)[...] &= np.uint32(0xFFFFE000)
        return b

    patch_w_r = tf32_round(patch_w)
    nc = _get_nc()
    in_maps = []
    for c in range(8):
        pT = np.zeros((D, 512), dtype=np.float32)
        pT[:, 0:196] = xr[2 * c].T
        pT[:, 256:452] = xr[2 * c + 1].T
        in_maps.append({
            "patches": tf32_round(pT), "h0T": h0T, "pw": patch_w_r,
            "wq_d": wq_d, "wk_d": wk_d, "w_rot": w_rot,
            "xi_d": xi_b, "xi_T": xi_Tb, "out_w": out_wb,
            "lng": ln_gamma, "lnb": ln_beta, "outg": out_gamma,
            "outbeta": out_beta, "out_b": out_b,
        })

    res = run_bass_kernel_spmd(nc, in_maps, core_ids=list(range(8)),
                               **_CACHE.get("run_kwargs", {}))
    if _CACHE.get("last_result_hook"):
        _CACHE["last_result_hook"](res)
    return np.concatenate([res.results[c]["out"] for c in range(8)], axis=0)
